# revision 1
# baseline (speedup 1.0000x reference)
"""Trainium2 Bass kernel for nn_MessageArMLP (GNN message passing).

message[e, r, a, c] = node_feat[sender[e], r, a, c]
                      * sigmoid(rc[e] @ W[group(a)])[c] * cutoff[e]

Strategy: shard the 120000 edges across 8 NeuronCores (15000 each).
Per core, edges are laid out 128-per-partition-column; a SWDGE dma_gather
fetches each edge's 5120-byte sender row from the replicated node_feat in
DRAM, the per-edge decay is computed with tiny PE matmuls + ACT sigmoid +
DVE broadcasts, the elementwise multiply runs on DVE in-place, and the
result streams back to DRAM. Memory-bound: ~154 MB of HBM traffic/core.
"""

import numpy as np
from contextlib import ExitStack

import concourse.bass as bass
import concourse.tile as tile
from concourse import bacc, mybir
from concourse.bass_utils import run_bass_kernel_spmd

dt = mybir.dt

# Problem constants (hardcoded per harness contract)
N_NODES = 10000
E_TOTAL = 120000
RADIAL = 8
ANG = 20
CH = 8
REMB = 8
ROW = RADIAL * ANG * CH     # 1280 f32 per node row (5120 B)
G = 4
GC = G * CH                 # 32
N_CORES = 8
E_SH = E_TOTAL // N_CORES   # 15000
COLS = -(-E_SH // 128)      # 118 columns of 128 edges
E_PAD = COLS * 128          # 15104
CHUNK_COLS = 8              # 1024 edges per chunk

# angular groups for MAX_L=3: sizes (l+1)(l+2)/2 = 1,3,6,10 -> starts 0,1,4,10
GROUP_SLOTS = [(0, 1), (1, 3), (4, 6), (10, 10)]


def _chunks(cols_total, chunk_cols):
    """Chunk decomposition: small first chunk primes the pipeline."""
    chunks = []
    first = min(2, cols_total)
    chunks.append((0, first))
    c = first
    while c < cols_total:
        w = min(chunk_cols, cols_total - c)
        chunks.append((c, w))
        c += w
    return chunks


def build_module(cols_total=COLS, chunk_cols=CHUNK_COLS, sender_bufs=4):
    e_pad = cols_total * 128
    nc = bacc.Bacc(
        "TRN2",
        target_bir_lowering=False,
        debug=False,
        enable_asserts=False,
        num_devices=N_CORES,
    )
    node = nc.dram_tensor(
        "node_feat", [N_NODES, ROW], dt.float32, kind="ExternalInput"
    ).ap()
    rct = nc.dram_tensor("rct", [REMB, e_pad], dt.float32, kind="ExternalInput").ap()
    cutf = nc.dram_tensor(
        "cutoff_t", [128, cols_total], dt.float32, kind="ExternalInput"
    ).ap()
    idx = nc.dram_tensor(
        "idx16", [128, e_pad // 16], dt.int16, kind="ExternalInput"
    ).ap()
    wt = nc.dram_tensor("wt", [REMB, GC], dt.float32, kind="ExternalInput").ap()
    msg = nc.dram_tensor("msg", [e_pad, ROW], dt.float32, kind="ExternalOutput").ap()

    chunks = _chunks(cols_total, chunk_cols)

    with tile.TileContext(nc) as tc:
        with ExitStack() as ctx:
            const_pool = ctx.enter_context(tc.tile_pool(name="const", bufs=1))
            sender_pool = ctx.enter_context(
                tc.tile_pool(name="sender", bufs=sender_bufs)
            )
            rct_pool = ctx.enter_context(tc.tile_pool(name="rct", bufs=2))
            dec_pool = ctx.enter_context(tc.tile_pool(name="dec", bufs=2))
            psum_pool = ctx.enter_context(tc.tile_pool(name="psum", bufs=2, space="PSUM"))

            # idx gates the first gather -> load it first on the sync ring;
            # wt/cutoff are needed later, load on the scalar ring
            idx_sb = const_pool.tile([128, e_pad // 16], dt.int16)
            nc.sync.dma_start(idx_sb[:], idx[:, :])
            wt_sb = const_pool.tile([REMB, GC], dt.float32)
            nc.scalar.dma_start(wt_sb[:], wt[:, :])
            cut_sb = const_pool.tile([128, cols_total], dt.float32)
            nc.scalar.dma_start(cut_sb[:], cutf[:, :])

            # partition-major: tile (p, j) -> DRAM row p*cols_total + j, so each
            # partition stores contiguous 5120*w byte runs
            msg_v = msg.rearrange("(p j) e -> p j e", j=cols_total)  # [128, cols, ROW]

            for ci, (c0, w) in enumerate(chunks):
                n_idx = w * 128
                sender = sender_pool.tile(
                    [128, chunk_cols, ROW], dt.float32, tag="sender"
                )
                nc.gpsimd.dma_gather(
                    out_ap=sender[:, :w, :],
                    in_ap=node[:, :],
                    idxs_ap=idx_sb[:, c0 * 8 : (c0 + w) * 8],
                    num_idxs=n_idx,
                    num_idxs_reg=n_idx,
                    elem_size=ROW,
                )

                rct_sb = rct_pool.tile([REMB, chunk_cols * 128], dt.float32, tag="rct")
                nc.scalar.dma_start(
                    rct_sb[:, :n_idx], rct[:, c0 * 128 : c0 * 128 + n_idx]
                )

                ps = psum_pool.tile([128, chunk_cols * GC], dt.float32, tag="ps")
                for j in range(w):
                    nc.tensor.matmul(
                        out=ps[:, j * GC : (j + 1) * GC],
                        lhsT=rct_sb[:, j * 128 : (j + 1) * 128],
                        rhs=wt_sb[:],
                        start=True,
                        stop=True,
                    )

                dec32 = dec_pool.tile([128, chunk_cols, GC], dt.float32, tag="dec32")
                nc.scalar.activation(
                    out=dec32[:, :w, :],
                    in_=ps[:, : w * GC],
                    func=mybir.ActivationFunctionType.Sigmoid,
                )

                # decay_a[p, j, a, c] = dec32[p, j, g(a), c] * cutoff[p, j]
                deca = dec_pool.tile([128, chunk_cols, ANG * CH], dt.float32, tag="deca")
                cut_b = cut_sb[:, c0 : c0 + w]
                for g, (s0, ns) in enumerate(GROUP_SLOTS):
                    nc.vector.tensor_mul(
                        out=deca[:, :w, s0 * CH : (s0 + ns) * CH].rearrange(
                            "p w (n c) -> p w n c", c=CH
                        ),
                        in0=dec32[:, :w, g * CH : (g + 1) * CH]
                        .unsqueeze(2)
                        .to_broadcast([128, w, ns, CH]),
                        in1=cut_b.unsqueeze(2)
                        .unsqueeze(3)
                        .to_broadcast([128, w, ns, CH]),
                    )

                # message = sender * decay_a (broadcast over r), in place
                sv = sender[:, :w, :].rearrange("p w (r ac) -> p w r ac", ac=ANG * CH)
                nc.vector.tensor_mul(
                    out=sv,
                    in0=sv,
                    in1=deca[:, :w, :]
                    .unsqueeze(2)
                    .to_broadcast([128, w, RADIAL, ANG * CH]),
                )

                nc.sync.dma_start(
                    out=msg_v[:, c0 : c0 + w, :], in_=sender[:, :w, :]
                )

    nc.compile()
    return nc


def make_in_maps(node_feat, radial_component, radial_cutoff_fn, weights, edge_index,
                 cols_total=COLS, chunk_cols=CHUNK_COLS, n_cores=N_CORES,
                 e_sh=E_SH):
    """Host-side sharding/layout prep. Only reorders/pads small tensors."""
    e_pad = cols_total * 128
    node_flat = np.ascontiguousarray(
        np.asarray(node_feat, dtype=np.float32).reshape(N_NODES, ROW)
    )
    wt = np.ascontiguousarray(
        np.asarray(weights, dtype=np.float32).transpose(1, 0, 2).reshape(REMB, GC)
    )
    senders = np.asarray(edge_index)[0]
    rc_all = np.asarray(radial_component, dtype=np.float32)
    cut_all = np.asarray(radial_cutoff_fn, dtype=np.float32)

    in_maps = []
    for i in range(n_cores):
        sl = slice(i * e_sh, (i + 1) * e_sh)
        idxs = np.zeros(e_pad, np.int16)
        idxs[:e_sh] = senders[sl].astype(np.int16)
        rc = np.zeros((e_pad, REMB), np.float32)
        rc[:e_sh] = rc_all[sl]
        cut = np.zeros(e_pad, np.float32)
        cut[:e_sh] = cut_all[sl]

        # partition-major layout: gather position (p, j) holds shard edge
        # p*cols_total + j (so stores write contiguous per-partition runs)
        idx_pm = idxs.reshape(128, cols_total)
        idx16 = np.zeros((128, e_pad // 16), np.int16)
        for c0, w in _chunks(cols_total, chunk_cols):
            # gather list position i = j_rel*128 + p
            seg = np.ascontiguousarray(idx_pm[:, c0 : c0 + w].T).reshape(-1)
            # wrapped [16, cols] block replicated to all 8 Q7-core stripes
            idx16[:, c0 * 8 : (c0 + w) * 8] = np.tile(seg.reshape(w * 8, 16).T, (8, 1))

        rct = np.ascontiguousarray(
            rc.reshape(128, cols_total, REMB)
            .transpose(1, 0, 2)
            .reshape(e_pad, REMB)
            .T
        )
        in_maps.append(
            {
                "node_feat": node_flat,
                "rct": rct,
                "cutoff_t": np.ascontiguousarray(cut.reshape(128, cols_total)),
                "idx16": idx16,
                "wt": wt,
            }
        )
    return in_maps


_nc_cache = None


def _get_module():
    global _nc_cache
    if _nc_cache is None:
        _nc_cache = build_module()
    return _nc_cache


def kernel(node_feat, radial_component, radial_cutoff_fn, weights, edge_index):
    nc = _get_module()
    in_maps = make_in_maps(
        node_feat, radial_component, radial_cutoff_fn, weights, edge_index
    )
    res = run_bass_kernel_spmd(nc, in_maps, core_ids=list(range(N_CORES)))
    outs = [r["msg"][:E_SH] for r in res.results]
    return np.concatenate(outs, 0).reshape(E_TOTAL, RADIAL, ANG, CH)



# revision 2
# speedup vs baseline: 1.7149x; 1.7149x over previous
"""Trainium2 Bass kernel for nn_MessageArMLP (GNN message passing).

message[e, r, a, c] = node_feat[sender[e], r, a, c]
                      * sigmoid(rc[e] @ W[group(a)])[c] * cutoff[e]

Strategy: shard the 120000 edges across 8 NeuronCores (15000 each).
Per core, edges are laid out 128-per-partition-column; a SWDGE dma_gather
fetches each edge's 2560-byte bf16 sender row from the replicated node_feat
in DRAM, the per-edge decay is computed with tiny PE matmuls + ACT sigmoid +
DVE broadcasts, the elementwise multiply runs on DVE in-place (bf16), and
the result streams back to DRAM as bf16 (host upcasts to f32; the 2e-2
rel-err budget dwarfs bf16's ~0.6% worst-case product error).
Memory-bound: ~78 MB of HBM traffic/core.
"""

import numpy as np
import ml_dtypes
from contextlib import ExitStack

import concourse.bass as bass
import concourse.tile as tile
from concourse import bacc, mybir
from concourse.bass_utils import run_bass_kernel_spmd

dt = mybir.dt

# Problem constants (hardcoded per harness contract)
N_NODES = 10000
E_TOTAL = 120000
RADIAL = 8
ANG = 20
CH = 8
REMB = 8
ROW = RADIAL * ANG * CH     # 1280 elems per node row (2560 B bf16)
G = 4
GC = G * CH                 # 32
N_CORES = 8
E_SH = E_TOTAL // N_CORES   # 15000
COLS = -(-E_SH // 128)      # 118 columns of 128 edges
E_PAD = COLS * 128          # 15104
CHUNK_COLS = 8              # 1024 edges per chunk

# angular groups for MAX_L=3: sizes (l+1)(l+2)/2 = 1,3,6,10 -> starts 0,1,4,10
GROUP_SLOTS = [(0, 1), (1, 3), (4, 6), (10, 10)]


def _chunks(cols_total, chunk_cols):
    """Chunk decomposition: small first chunk primes the pipeline."""
    chunks = []
    first = min(2, cols_total)
    chunks.append((0, first))
    c = first
    while c < cols_total:
        w = min(chunk_cols, cols_total - c)
        chunks.append((c, w))
        c += w
    return chunks


def build_module(cols_total=COLS, chunk_cols=CHUNK_COLS, sender_bufs=4):
    e_pad = cols_total * 128
    nc = bacc.Bacc(
        "TRN2",
        target_bir_lowering=False,
        debug=False,
        enable_asserts=False,
        num_devices=N_CORES,
    )
    node = nc.dram_tensor(
        "node_feat", [N_NODES, ROW], dt.bfloat16, kind="ExternalInput"
    ).ap()
    rct = nc.dram_tensor("rct", [REMB, e_pad], dt.float32, kind="ExternalInput").ap()
    cutf = nc.dram_tensor(
        "cutoff_t", [128, cols_total], dt.float32, kind="ExternalInput"
    ).ap()
    idx = nc.dram_tensor(
        "idx16", [128, e_pad // 16], dt.int16, kind="ExternalInput"
    ).ap()
    wt = nc.dram_tensor("wt", [REMB, GC], dt.float32, kind="ExternalInput").ap()
    msg = nc.dram_tensor("msg", [e_pad, ROW], dt.bfloat16, kind="ExternalOutput").ap()

    chunks = _chunks(cols_total, chunk_cols)

    with tile.TileContext(nc) as tc:
        with ExitStack() as ctx:
            const_pool = ctx.enter_context(tc.tile_pool(name="const", bufs=1))
            sender_pool = ctx.enter_context(
                tc.tile_pool(name="sender", bufs=sender_bufs)
            )
            rct_pool = ctx.enter_context(tc.tile_pool(name="rct", bufs=2))
            dec_pool = ctx.enter_context(tc.tile_pool(name="dec", bufs=2))
            psum_pool = ctx.enter_context(tc.tile_pool(name="psum", bufs=2, space="PSUM"))

            # idx gates the first gather -> load it first on the sync ring;
            # wt/cutoff are needed later, load on the scalar ring
            idx_sb = const_pool.tile([128, e_pad // 16], dt.int16)
            nc.sync.dma_start(idx_sb[:], idx[:, :])
            wt_sb = const_pool.tile([REMB, GC], dt.float32)
            nc.scalar.dma_start(wt_sb[:], wt[:, :])
            cut_sb = const_pool.tile([128, cols_total], dt.float32)
            nc.scalar.dma_start(cut_sb[:], cutf[:, :])

            # partition-major: tile (p, j) -> DRAM row p*cols_total + j, so each
            # partition stores contiguous 2560*w byte runs
            msg_v = msg.rearrange("(p j) e -> p j e", j=cols_total)  # [128, cols, ROW]

            for ci, (c0, w) in enumerate(chunks):
                n_idx = w * 128
                sender = sender_pool.tile(
                    [128, chunk_cols, ROW], dt.bfloat16, tag="sender"
                )
                nc.gpsimd.dma_gather(
                    out_ap=sender[:, :w, :],
                    in_ap=node[:, :],
                    idxs_ap=idx_sb[:, c0 * 8 : (c0 + w) * 8],
                    num_idxs=n_idx,
                    num_idxs_reg=n_idx,
                    elem_size=ROW,
                )

                rct_sb = rct_pool.tile([REMB, chunk_cols * 128], dt.float32, tag="rct")
                nc.scalar.dma_start(
                    rct_sb[:, :n_idx], rct[:, c0 * 128 : c0 * 128 + n_idx]
                )

                ps = psum_pool.tile([128, chunk_cols * GC], dt.float32, tag="ps")
                for j in range(w):
                    nc.tensor.matmul(
                        out=ps[:, j * GC : (j + 1) * GC],
                        lhsT=rct_sb[:, j * 128 : (j + 1) * 128],
                        rhs=wt_sb[:],
                        start=True,
                        stop=True,
                    )

                dec32 = dec_pool.tile([128, chunk_cols, GC], dt.float32, tag="dec32")
                nc.scalar.activation(
                    out=dec32[:, :w, :],
                    in_=ps[:, : w * GC],
                    func=mybir.ActivationFunctionType.Sigmoid,
                )

                # decay_a[p, j, a, c] = dec32[p, j, g(a), c] * cutoff[p, j]
                deca = dec_pool.tile([128, chunk_cols, ANG * CH], dt.bfloat16, tag="deca")
                cut_b = cut_sb[:, c0 : c0 + w]
                for g, (s0, ns) in enumerate(GROUP_SLOTS):
                    nc.vector.tensor_mul(
                        out=deca[:, :w, s0 * CH : (s0 + ns) * CH].rearrange(
                            "p w (n c) -> p w n c", c=CH
                        ),
                        in0=dec32[:, :w, g * CH : (g + 1) * CH]
                        .unsqueeze(2)
                        .to_broadcast([128, w, ns, CH]),
                        in1=cut_b.unsqueeze(2)
                        .unsqueeze(3)
                        .to_broadcast([128, w, ns, CH]),
                    )

                # message = sender * decay_a (broadcast over r), in place
                sv = sender[:, :w, :].rearrange("p w (r ac) -> p w r ac", ac=ANG * CH)
                nc.vector.tensor_mul(
                    out=sv,
                    in0=sv,
                    in1=deca[:, :w, :]
                    .unsqueeze(2)
                    .to_broadcast([128, w, RADIAL, ANG * CH]),
                )

                nc.sync.dma_start(
                    out=msg_v[:, c0 : c0 + w, :], in_=sender[:, :w, :]
                )

    nc.compile()
    return nc


def _f32_to_bf16(a):
    """Round-to-nearest-even f32 -> bf16, as raw uint16-backed bf16 array."""
    u = np.ascontiguousarray(a, dtype=np.float32).view(np.uint32)
    r = ((u + 0x7FFF + ((u >> 16) & 1)) >> 16).astype(np.uint16)
    return r.view(ml_dtypes.bfloat16)


def make_in_maps(node_feat, radial_component, radial_cutoff_fn, weights, edge_index,
                 cols_total=COLS, chunk_cols=CHUNK_COLS, n_cores=N_CORES,
                 e_sh=E_SH):
    """Host-side sharding/layout prep. Only reorders/pads/converts dtype."""
    e_pad = cols_total * 128
    node_bf = _f32_to_bf16(np.asarray(node_feat).reshape(N_NODES, ROW))
    wt = np.ascontiguousarray(
        np.asarray(weights, dtype=np.float32).transpose(1, 0, 2).reshape(REMB, GC)
    )
    senders = np.asarray(edge_index)[0]
    rc_all = np.asarray(radial_component, dtype=np.float32)
    cut_all = np.asarray(radial_cutoff_fn, dtype=np.float32)

    in_maps = []
    for i in range(n_cores):
        sl = slice(i * e_sh, (i + 1) * e_sh)
        idxs = np.zeros(e_pad, np.int16)
        idxs[:e_sh] = senders[sl].astype(np.int16)
        rc = np.zeros((e_pad, REMB), np.float32)
        rc[:e_sh] = rc_all[sl]
        cut = np.zeros(e_pad, np.float32)
        cut[:e_sh] = cut_all[sl]

        # partition-major layout: gather position (p, j) holds shard edge
        # p*cols_total + j (so stores write contiguous per-partition runs)
        idx_pm = idxs.reshape(128, cols_total)
        idx16 = np.zeros((128, e_pad // 16), np.int16)
        for c0, w in _chunks(cols_total, chunk_cols):
            # gather list position i = j_rel*128 + p
            seg = np.ascontiguousarray(idx_pm[:, c0 : c0 + w].T).reshape(-1)
            # wrapped [16, cols] block replicated to all 8 Q7-core stripes
            idx16[:, c0 * 8 : (c0 + w) * 8] = np.tile(seg.reshape(w * 8, 16).T, (8, 1))

        rct = np.ascontiguousarray(
            rc.reshape(128, cols_total, REMB)
            .transpose(1, 0, 2)
            .reshape(e_pad, REMB)
            .T
        )
        in_maps.append(
            {
                "node_feat": node_bf,
                "rct": rct,
                "cutoff_t": np.ascontiguousarray(cut.reshape(128, cols_total)),
                "idx16": idx16,
                "wt": wt,
            }
        )
    return in_maps


_nc_cache = None


def _get_module():
    global _nc_cache
    if _nc_cache is None:
        _nc_cache = build_module()
    return _nc_cache


def _msg_to_f32(msg):
    """Exact bf16 -> f32 upcast of the device output."""
    u = np.ascontiguousarray(msg).view(np.uint16).astype(np.uint32) << 16
    return u.view(np.float32)


def kernel(node_feat, radial_component, radial_cutoff_fn, weights, edge_index):
    nc = _get_module()
    in_maps = make_in_maps(
        node_feat, radial_component, radial_cutoff_fn, weights, edge_index
    )
    res = run_bass_kernel_spmd(nc, in_maps, core_ids=list(range(N_CORES)))
    outs = [_msg_to_f32(r["msg"][:E_SH]) for r in res.results]
    return np.concatenate(outs, 0).reshape(E_TOTAL, RADIAL, ANG, CH)
